# revision 5
# baseline (speedup 1.0000x reference)
"""Trainium2 Bass kernel for the MANN (memory-augmented NN) problem.

Reference computation (per batch of B=2048 samples):
    h        = tanh(x @ W_h + b_h)                  [B, 512]
    gate     = x @ W_g + b_g                        [B, 1]
    read_key = tanh(h @ W_k + b_k)                  [B, 64]
    kn       = read_key / (||read_key|| + eps)
    Mn       = M / (||M_row|| + eps)                [65536, 64]
    sim      = kn @ Mn.T                            [B, 65536]
    w_read   = softmax(sim, axis=-1)
    r        = w_read @ M                           [B, 64]
    out      = (concat(h, r) @ W_out + b_out)[:, 0] [B]
    returns (out, h[-1], gate[-1], w_read[-1])

Strategy: data-parallel over batch across 8 NeuronCores (256 samples each).
On each core everything is computed in a "transposed" layout (features on
partitions, batch on the free axis) so that the streaming pass over the
65536-location memory needs NO on-chip transposes:

    simT_chunk [128 locs, 256 B] = matmul(lhsT=MnT_chunk [64, 128],
                                          rhs =knT       [64, 256])
    w2 = exp(simT)                          (scalar engine, PSUM -> SBUF)
    rT [65, 256] += matmul(lhsT=[M_chunk | ones] [128, 65], rhs=w2 [128, 256])

The ones column folded into the r-matmul accumulates the softmax
denominator Z in row 64 of the same PSUM accumulator.  Cosine similarity
is bounded in [-1, 1], so exp() needs no running-max for stability.
"""

import numpy as np

NCORES = 8
B = 2048
IN_DIM = 512
CTRL = 512
N_LOC = 65536
LOC = 64
EPS = 1e-8

BC = B // NCORES          # 256 batch per core
KT = IN_DIM // 128        # 4 input k-tiles
CT = CTRL // 128          # 4 ctrl tiles
NCHUNK = N_LOC // 128     # 512 location chunks
GROUP = 4                 # chunks per exp() activation call (psum group)
SUPER = 16                # chunks per DMA superblock
NSB = NCHUNK // SUPER     # 32 superblocks

_CACHE = {}


def _build_program():
    import concourse.mybir as mybir
    import concourse.tile as tile
    from concourse import bacc
    from contextlib import ExitStack

    f32 = mybir.dt.float32
    f32r = mybir.dt.float32r
    AF = mybir.ActivationFunctionType

    nc = bacc.Bacc("TRN2", target_bir_lowering=False, debug=False)

    # ---------------- DRAM I/O (per-core shapes; host pre-swizzled) ----------
    xT = nc.dram_tensor("xT", [128, KT, BC], f32r, kind="ExternalInput")
    Wh = nc.dram_tensor("Wh", [128, KT, CTRL], f32r, kind="ExternalInput")
    bh = nc.dram_tensor("bh", [128, CT], f32, kind="ExternalInput")
    Wg = nc.dram_tensor("Wg", [128, KT], f32r, kind="ExternalInput")
    bg = nc.dram_tensor("bg", [1, 1], f32, kind="ExternalInput")
    Wk = nc.dram_tensor("Wk", [128, KT, LOC], f32r, kind="ExternalInput")
    bk = nc.dram_tensor("bk", [LOC, 1], f32, kind="ExternalInput")
    MnT = nc.dram_tensor("MnT", [LOC, N_LOC], f32r, kind="ExternalInput")
    Mo = nc.dram_tensor("Mo", [128, NCHUNK, LOC + 1], f32r, kind="ExternalInput")
    WoH = nc.dram_tensor("WoH", [128, CT], f32r, kind="ExternalInput")
    WoR = nc.dram_tensor("WoR", [LOC, 1], f32r, kind="ExternalInput")
    bo = nc.dram_tensor("bo", [1, 1], f32, kind="ExternalInput")

    out_d = nc.dram_tensor("out", [1, BC], f32, kind="ExternalOutput")
    hl_d = nc.dram_tensor("hl", [128, CT], f32, kind="ExternalOutput")
    gt_d = nc.dram_tensor("gt", [1, BC], f32, kind="ExternalOutput")
    wl_d = nc.dram_tensor("wl", [128, NCHUNK], f32, kind="ExternalOutput")

    def r(ap):
        return ap

    with tile.TileContext(nc) as tc, ExitStack() as ctx:
        const = ctx.enter_context(tc.tile_pool(name="const", bufs=1))
        mnp = ctx.enter_context(tc.tile_pool(name="mnp", bufs=2))
        mop = ctx.enter_context(tc.tile_pool(name="mop", bufs=2))
        w2p = ctx.enter_context(tc.tile_pool(name="w2p", bufs=2))
        ps_sim = ctx.enter_context(tc.tile_pool(name="ps_sim", bufs=2, space="PSUM"))
        ps_r = ctx.enter_context(tc.tile_pool(name="ps_r", bufs=1, space="PSUM"))
        ps_misc = ctx.enter_context(tc.tile_pool(name="ps_misc", bufs=3, space="PSUM"))

        # ------------- constants / weights to SBUF -------------
        xT_sb = const.tile([128, KT, BC], f32r)
        nc.sync.dma_start(out=xT_sb[:], in_=xT[:])
        Wh_sb = const.tile([128, KT, CTRL], f32r)
        nc.sync.dma_start(out=Wh_sb[:], in_=Wh[:])
        bh_sb = const.tile([128, CT], f32)
        nc.sync.dma_start(out=bh_sb[:], in_=bh[:])
        Wg_sb = const.tile([128, KT], f32r)
        nc.sync.dma_start(out=Wg_sb[:], in_=Wg[:])
        bg_sb = const.tile([1, 1], f32)
        nc.sync.dma_start(out=bg_sb[:], in_=bg[:])
        Wk_sb = const.tile([128, KT, LOC], f32r)
        nc.sync.dma_start(out=Wk_sb[:], in_=Wk[:])
        bk_sb = const.tile([LOC, 1], f32)
        nc.sync.dma_start(out=bk_sb[:], in_=bk[:])
        WoH_sb = const.tile([128, CT], f32r)
        nc.sync.dma_start(out=WoH_sb[:], in_=WoH[:])
        WoR_sb = const.tile([LOC, 1], f32r)
        nc.sync.dma_start(out=WoR_sb[:], in_=WoR[:])
        bo_sb = const.tile([1, 1], f32)
        nc.sync.dma_start(out=bo_sb[:], in_=bo[:])

        ones_sb = const.tile([128, 128], f32)
        nc.vector.memset(ones_sb[:], 1.0)

        # ------------- controller: hT = tanh(Wh^T xT + bh) [512c, 256b] -----
        hT_sb = const.tile([128, CT, BC], f32r)
        for ct in range(CT):
            ps_h = ps_misc.tile([128, BC], f32, tag="m")
            for k in range(KT):
                nc.tensor.matmul(
                    ps_h[:],
                    r(Wh_sb[:, k, ct * 128:(ct + 1) * 128]),
                    r(xT_sb[:, k, :]),
                    start=(k == 0), stop=(k == KT - 1),
                )
            nc.scalar.activation(
                out=hT_sb[:, ct, :], in_=ps_h[:], func=AF.Tanh,
                bias=bh_sb[:, ct:ct + 1], scale=1.0,
            )

        # ------------- gate = x W_g + b_g  -> [1, 256] ----------------------
        ps_g = ps_misc.tile([1, BC], f32, tag="m")
        for k in range(KT):
            nc.tensor.matmul(
                ps_g[:], r(Wg_sb[:, k:k + 1]), r(xT_sb[:, k, :]),
                start=(k == 0), stop=(k == KT - 1),
            )
        gt_sb = const.tile([1, BC], f32)
        nc.scalar.activation(out=gt_sb[:], in_=ps_g[:], func=AF.Identity,
                             bias=bg_sb[0:1, 0:1], scale=1.0)
        nc.sync.dma_start(out=gt_d[:], in_=gt_sb[:])

        # ------------- read key: rkT = tanh(Wk^T hT + bk) [64, 256] ---------
        ps_rk = ps_misc.tile([LOC, BC], f32, tag="m")
        for k in range(KT):
            nc.tensor.matmul(
                ps_rk[:], r(Wk_sb[:, k, :]), r(hT_sb[:, k, :]),
                start=(k == 0), stop=(k == KT - 1),
            )
        rkT_sb = const.tile([LOC, BC], f32)
        nc.scalar.activation(out=rkT_sb[:], in_=ps_rk[:], func=AF.Tanh,
                             bias=bk_sb[:], scale=1.0)

        # ------------- kn = rk / ||rk||  (per-sample norm) ------------------
        rksq_sb = const.tile([LOC, BC], f32)
        nc.vector.tensor_mul(rksq_sb[:], rkT_sb[:], rkT_sb[:])
        ps_ss = ps_misc.tile([1, BC], f32, tag="m")  # sum over 64 feats via ones-matmul
        nc.tensor.matmul(ps_ss[:], r(ones_sb[0:LOC, 0:1]), r(rksq_sb[:]),
                         start=True, stop=True)
        lnss_sb = const.tile([1, BC], f32)
        nc.scalar.activation(out=lnss_sb[:], in_=ps_ss[:], func=AF.Ln)
        invn_sb = const.tile([1, BC], f32)  # 1/||rk|| = exp(-0.5 ln(ss))
        nc.scalar.activation(out=invn_sb[:], in_=lnss_sb[:], func=AF.Exp,
                             scale=-0.5)
        # broadcast 1/||rk|| across the 64 feature partitions via K=1 matmul
        ps_bc = ps_misc.tile([LOC, BC], f32, tag="m")
        nc.tensor.matmul(ps_bc[:], r(ones_sb[0:1, 0:LOC]), r(invn_sb[:]),
                         start=True, stop=True)
        knT_sb = const.tile([LOC, BC], f32r)
        nc.vector.tensor_mul(knT_sb[:], rkT_sb[:], ps_bc[:])

        # ------------- streaming pass over the 65536 memory locations -------
        wlast_sb = const.tile([128, NCHUNK], f32)   # unnormalized w of sample BC-1
        ps_rT = ps_r.tile([LOC + 1, BC], f32)       # rows 0..63: r^T; row 64: Z
        for sb in range(NSB):
            c0 = sb * SUPER
            mn_t = mnp.tile([LOC, SUPER, 128], f32r)
            nc.sync.dma_start(
                out=mn_t[:],
                in_=MnT[:, c0 * 128:(c0 + SUPER) * 128].rearrange(
                    "f (s c) -> f s c", s=SUPER),
            )
            mo_t = mop.tile([128, SUPER, LOC + 1], f32r)
            nc.sync.dma_start(out=mo_t[:], in_=Mo[:, c0:c0 + SUPER, :])
            w2_t = w2p.tile([128, SUPER, BC], f32r)
            for g in range(SUPER // GROUP):
                ps_s = ps_sim.tile([128, GROUP, BC], f32)
                for j in range(GROUP):
                    nc.tensor.matmul(
                        ps_s[:, j, :],
                        r(mn_t[:, g * GROUP + j, :]),
                        r(knT_sb[:]),
                        start=True, stop=True,
                    )
                nc.scalar.activation(
                    out=w2_t[:, g * GROUP:(g + 1) * GROUP, :],
                    in_=ps_s[:], func=AF.Exp,
                )
            for j in range(SUPER):
                c = c0 + j
                nc.tensor.matmul(
                    ps_rT[:], r(mo_t[:, j, :]), r(w2_t[:, j, :]),
                    start=(c == 0), stop=(c == NCHUNK - 1),
                )
            nc.vector.tensor_copy(out=wlast_sb[:, c0:c0 + SUPER],
                                  in_=w2_t[:, :, BC - 1])

        # ------------- epilogue ---------------------------------------------
        # zw[64, b] = 1/Z_b  (stays on partition 64, where Z landed)
        zw_sb = const.tile([128, BC], f32)
        nc.vector.reciprocal(out=zw_sb[64:65, :], in_=ps_rT[LOC:LOC + 1, :])

        # out = hT^T WoH + (rT^T WoR) / Z + bo, all as [1, 256] rows
        racc_sb = const.tile([LOC, BC], f32r)
        nc.vector.tensor_copy(out=racc_sb[:], in_=ps_rT[0:LOC, :])
        ps_oh = ps_misc.tile([1, BC], f32, tag="m")
        for k in range(CT):
            nc.tensor.matmul(ps_oh[:], r(WoH_sb[:, k:k + 1]), r(hT_sb[:, k, :]),
                             start=(k == 0), stop=(k == CT - 1))
        ps_or = ps_misc.tile([1, BC], f32, tag="m")
        nc.tensor.matmul(ps_or[:], r(WoR_sb[:]), r(racc_sb[:]),
                         start=True, stop=True)
        # broadcast 1/Z from partition 64 to partition 0 (row 0 of [64, BC])
        ps_zb = ps_misc.tile([LOC, BC], f32, tag="m")
        nc.tensor.matmul(ps_zb[:], r(ones_sb[64:65, 0:LOC]), r(zw_sb[64:65, :]),
                         start=True, stop=True)
        zb_sb = const.tile([LOC, BC], f32)
        nc.vector.tensor_copy(out=zb_sb[:], in_=ps_zb[:])
        t1_sb = const.tile([1, BC], f32)
        nc.vector.tensor_mul(t1_sb[:], zb_sb[0:1, :], ps_or[:])
        t2_sb = const.tile([1, BC], f32)
        nc.vector.tensor_add(t2_sb[:], t1_sb[:], ps_oh[:])
        outv_sb = const.tile([1, BC], f32)
        nc.vector.tensor_scalar_add(outv_sb[:], t2_sb[:], bo_sb[0:1, 0:1])
        nc.sync.dma_start(out=out_d[:], in_=outv_sb[:])

        # h[-1]: column BC-1 of hT
        hl_sb = const.tile([128, CT], f32)
        nc.vector.tensor_copy(out=hl_sb[:], in_=hT_sb[:, :, BC - 1])
        nc.sync.dma_start(out=hl_d[:], in_=hl_sb[:])

        # w_read[-1]: wlast * (1/Z[BC-1]) broadcast to all 128 partitions
        ps_zl = ps_misc.tile([128, 1], f32, tag="m")
        nc.tensor.matmul(ps_zl[:], r(ones_sb[64:65, :]),
                         r(zw_sb[64:65, BC - 1:BC]), start=True, stop=True)
        zl_sb = const.tile([128, 1], f32)
        nc.vector.tensor_copy(out=zl_sb[:], in_=ps_zl[:])
        wlf_sb = const.tile([128, NCHUNK], f32)
        nc.vector.tensor_scalar_mul(wlf_sb[:], wlast_sb[:], zl_sb[:, 0:1])
        nc.sync.dma_start(out=wl_d[:], in_=wlf_sb[:])

    nc.compile()
    return nc


def _get_program():
    if "nc" not in _CACHE:
        _CACHE["nc"] = _build_program()
    return _CACHE["nc"]


def _prep_in_maps(inputs):
    return _prep(**{k: np.asarray(v) for k, v in inputs.items()})


def _prep(x, W_h, b_h, W_g, b_g, W_k, b_k, M, W_out, b_out):
    x = np.ascontiguousarray(np.asarray(x, dtype=np.float32))
    W_h = np.asarray(W_h, dtype=np.float32)
    b_h = np.asarray(b_h, dtype=np.float32)
    W_g = np.asarray(W_g, dtype=np.float32)
    b_g = np.asarray(b_g, dtype=np.float32)
    W_k = np.asarray(W_k, dtype=np.float32)
    b_k = np.asarray(b_k, dtype=np.float32)
    M = np.ascontiguousarray(np.asarray(M, dtype=np.float32))
    W_out = np.asarray(W_out, dtype=np.float32)
    b_out = np.asarray(b_out, dtype=np.float32)

    # ---- host-side layout prep (weight swizzles for SBUF-friendly DMA) ----
    norms = np.linalg.norm(M, axis=1, keepdims=True)
    MnT = np.ascontiguousarray((M / (norms + EPS)).T)           # [64, 65536]
    Mo = np.concatenate([M, np.ones((N_LOC, 1), np.float32)], axis=1)
    Mo = np.ascontiguousarray(
        Mo.reshape(NCHUNK, 128, LOC + 1).transpose(1, 0, 2))    # [128, 512, 65]

    Wh_p = np.ascontiguousarray(W_h.reshape(KT, 128, CTRL).transpose(1, 0, 2))
    bh_p = np.ascontiguousarray(b_h.reshape(CT, 128).T)
    Wg_p = np.ascontiguousarray(W_g[:, 0].reshape(KT, 128).T)
    bg_p = b_g.reshape(1, 1)
    Wk_p = np.ascontiguousarray(W_k.reshape(KT, 128, LOC).transpose(1, 0, 2))
    bk_p = b_k.reshape(LOC, 1)
    WoH_p = np.ascontiguousarray(W_out[:CTRL, 0].reshape(CT, 128).T)
    WoR_p = np.ascontiguousarray(W_out[CTRL:, 0:1])
    bo_p = b_out.reshape(1, 1)

    shared = dict(Wh=Wh_p, bh=bh_p, Wg=Wg_p, bg=bg_p, Wk=Wk_p, bk=bk_p,
                  MnT=MnT, Mo=Mo, WoH=WoH_p, WoR=WoR_p, bo=bo_p)
    in_maps = []
    for c in range(NCORES):
        xc = x[c * BC:(c + 1) * BC]                              # [256, 512]
        xT_p = np.ascontiguousarray(
            xc.T.reshape(KT, 128, BC).transpose(1, 0, 2))        # [128, 4, 256]
        in_maps.append(dict(shared, xT=xT_p))
    return in_maps


def kernel(x, W_h, b_h, W_g, b_g, W_k, b_k, M, W_out, b_out):
    from concourse.bass_utils import run_bass_kernel_spmd

    in_maps = _prep(x, W_h, b_h, W_g, b_g, W_k, b_k, M, W_out, b_out)
    nc = _get_program()
    res = run_bass_kernel_spmd(nc, in_maps, core_ids=list(range(NCORES))).results

    output = np.concatenate([res[c]["out"][0] for c in range(NCORES)])
    h_last = np.ascontiguousarray(res[NCORES - 1]["hl"].T).reshape(CTRL)
    gate_last = res[NCORES - 1]["gt"][0, BC - 1:BC].copy()
    w_read_last = np.ascontiguousarray(res[NCORES - 1]["wl"].T).reshape(N_LOC)
    return (output.astype(np.float32), h_last.astype(np.float32),
            gate_last.astype(np.float32), w_read_last.astype(np.float32))


# revision 8
# speedup vs baseline: 1.6055x; 1.6055x over previous
"""Trainium2 Bass kernel for the MANN (memory-augmented NN) problem.

Reference computation (per batch of B=2048 samples):
    h        = tanh(x @ W_h + b_h)                  [B, 512]
    gate     = x @ W_g + b_g                        [B, 1]
    read_key = tanh(h @ W_k + b_k)                  [B, 64]
    kn       = read_key / (||read_key|| + eps)
    Mn       = M / (||M_row|| + eps)                [65536, 64]
    sim      = kn @ Mn.T                            [B, 65536]
    w_read   = softmax(sim, axis=-1)
    r        = w_read @ M                           [B, 64]
    out      = (concat(h, r) @ W_out + b_out)[:, 0] [B]
    returns (out, h[-1], gate[-1], w_read[-1])

Strategy: data-parallel over batch across 8 NeuronCores (256 samples each).
On each core everything is computed in a "transposed" layout (features on
partitions, batch on the free axis) so that the streaming pass over the
65536-location memory needs NO on-chip transposes:

    simT_chunk [128 locs, 256 B] = matmul(lhsT=MnT_chunk [64, 128],
                                          rhs =knT       [64, 256])
    w2 = exp(simT)                          (scalar engine, PSUM -> SBUF)
    rT [65, 256] += matmul(lhsT=[M_chunk | ones] [128, 65], rhs=w2 [128, 256])

The ones column folded into the r-matmul accumulates the softmax
denominator Z in row 64 of the same PSUM accumulator.  Cosine similarity
is bounded in [-1, 1], so exp() needs no running-max for stability.
"""

import numpy as np

NCORES = 8
B = 2048
IN_DIM = 512
CTRL = 512
N_LOC = 65536
LOC = 64
EPS = 1e-8

BC = B // NCORES          # 256 batch per core
KT = IN_DIM // 128        # 4 input k-tiles
CT = CTRL // 128          # 4 ctrl tiles
NCHUNK = N_LOC // 128     # 512 location chunks
GROUP = 4                 # chunks per exp() activation call (psum group)
SUPER = 16                # chunks per DMA superblock
NSB = NCHUNK // SUPER     # 32 superblocks

_CACHE = {}
DEBUG_TAPS = False


def _build_program():
    import concourse.mybir as mybir
    import concourse.tile as tile
    from concourse import bacc
    from contextlib import ExitStack

    f32 = mybir.dt.float32
    f32r = mybir.dt.float32r
    bf16 = mybir.dt.bfloat16
    AF = mybir.ActivationFunctionType

    nc = bacc.Bacc("TRN2", target_bir_lowering=False, debug=False)

    # ---------------- DRAM I/O (per-core shapes; host pre-swizzled) ----------
    xT = nc.dram_tensor("xT", [128, KT, BC], f32, kind="ExternalInput")
    Wh = nc.dram_tensor("Wh", [128, KT, CTRL], f32, kind="ExternalInput")
    bh = nc.dram_tensor("bh", [128, CT], f32, kind="ExternalInput")
    Wg = nc.dram_tensor("Wg", [128, KT], f32, kind="ExternalInput")
    bg = nc.dram_tensor("bg", [1, 1], f32, kind="ExternalInput")
    Wk = nc.dram_tensor("Wk", [128, KT, LOC], f32, kind="ExternalInput")
    bk = nc.dram_tensor("bk", [LOC, 1], f32, kind="ExternalInput")
    MnT = nc.dram_tensor("MnT", [LOC, N_LOC], bf16, kind="ExternalInput")
    Mo = nc.dram_tensor("Mo", [128, NCHUNK, LOC + 1], bf16, kind="ExternalInput")
    WoH = nc.dram_tensor("WoH", [128, CT], f32, kind="ExternalInput")
    WoR = nc.dram_tensor("WoR", [LOC, 1], f32, kind="ExternalInput")
    bo = nc.dram_tensor("bo", [1, 1], f32, kind="ExternalInput")

    out_d = nc.dram_tensor("out", [1, BC], f32, kind="ExternalOutput")
    if DEBUG_TAPS:
        dbg_kn = nc.dram_tensor("dbg_kn", [LOC, BC], f32, kind="ExternalOutput")
        dbg_mn = nc.dram_tensor("dbg_mn", [LOC, 128], f32, kind="ExternalOutput")
        dbg_mo = nc.dram_tensor("dbg_mo", [128, LOC + 1], f32, kind="ExternalOutput")
        dbg_sim = nc.dram_tensor("dbg_sim", [128, BC], f32, kind="ExternalOutput")
        dbg_w2 = nc.dram_tensor("dbg_w2", [128, BC], f32, kind="ExternalOutput")
        dbg_racc = nc.dram_tensor("dbg_racc", [LOC + 1, BC], f32, kind="ExternalOutput")
    hl_d = nc.dram_tensor("hl", [128, CT], f32, kind="ExternalOutput")
    gt_d = nc.dram_tensor("gt", [1, BC], f32, kind="ExternalOutput")
    wl_d = nc.dram_tensor("wl", [128, NCHUNK], f32, kind="ExternalOutput")

    def r(ap):
        return ap

    with tile.TileContext(nc) as tc, ExitStack() as ctx:
        const = ctx.enter_context(tc.tile_pool(name="const", bufs=1))
        mnp = ctx.enter_context(tc.tile_pool(name="mnp", bufs=2))
        mop = ctx.enter_context(tc.tile_pool(name="mop", bufs=2))
        w2p = ctx.enter_context(tc.tile_pool(name="w2p", bufs=2))
        ps_sim = ctx.enter_context(tc.tile_pool(name="ps_sim", bufs=2, space="PSUM"))
        ps_r = ctx.enter_context(tc.tile_pool(name="ps_r", bufs=1, space="PSUM"))
        ps_misc = ctx.enter_context(tc.tile_pool(name="ps_misc", bufs=3, space="PSUM"))

        # ------------- constants / weights to SBUF -------------
        xT_sb = const.tile([128, KT, BC], f32)
        nc.sync.dma_start(out=xT_sb[:], in_=xT[:])
        Wh_sb = const.tile([128, KT, CTRL], f32)
        nc.sync.dma_start(out=Wh_sb[:], in_=Wh[:])
        bh_sb = const.tile([128, CT], f32)
        nc.sync.dma_start(out=bh_sb[:], in_=bh[:])
        Wg_sb = const.tile([128, KT], f32)
        nc.sync.dma_start(out=Wg_sb[:], in_=Wg[:])
        bg_sb = const.tile([1, 1], f32)
        nc.sync.dma_start(out=bg_sb[:], in_=bg[:])
        Wk_sb = const.tile([128, KT, LOC], f32)
        nc.sync.dma_start(out=Wk_sb[:], in_=Wk[:])
        bk_sb = const.tile([LOC, 1], f32)
        nc.sync.dma_start(out=bk_sb[:], in_=bk[:])
        WoH_sb = const.tile([128, CT], f32)
        nc.sync.dma_start(out=WoH_sb[:], in_=WoH[:])
        WoR_sb = const.tile([LOC, 1], f32)
        nc.sync.dma_start(out=WoR_sb[:], in_=WoR[:])
        bo_sb = const.tile([1, 1], f32)
        nc.sync.dma_start(out=bo_sb[:], in_=bo[:])

        ones_sb = const.tile([128, 128], f32)
        nc.vector.memset(ones_sb[:], 1.0)

        # ------------- controller: hT = tanh(Wh^T xT + bh) [512c, 256b] -----
        hT_sb = const.tile([128, CT, BC], f32)
        for ct in range(CT):
            ps_h = ps_misc.tile([128, BC], f32, tag="m")
            for k in range(KT):
                nc.tensor.matmul(
                    ps_h[:],
                    r(Wh_sb[:, k, ct * 128:(ct + 1) * 128]),
                    r(xT_sb[:, k, :]),
                    start=(k == 0), stop=(k == KT - 1),
                )
            nc.scalar.activation(
                out=hT_sb[:, ct, :], in_=ps_h[:], func=AF.Tanh,
                bias=bh_sb[:, ct:ct + 1], scale=1.0,
            )

        # ------------- gate = x W_g + b_g  -> [1, 256] ----------------------
        ps_g = ps_misc.tile([1, BC], f32, tag="m")
        for k in range(KT):
            nc.tensor.matmul(
                ps_g[:], r(Wg_sb[:, k:k + 1]), r(xT_sb[:, k, :]),
                start=(k == 0), stop=(k == KT - 1),
            )
        gt_sb = const.tile([1, BC], f32)
        nc.scalar.activation(out=gt_sb[:], in_=ps_g[:], func=AF.Identity,
                             bias=bg_sb[0:1, 0:1], scale=1.0)
        nc.sync.dma_start(out=gt_d[:], in_=gt_sb[:])

        # ------------- read key: rkT = tanh(Wk^T hT + bk) [64, 256] ---------
        ps_rk = ps_misc.tile([LOC, BC], f32, tag="m")
        for k in range(KT):
            nc.tensor.matmul(
                ps_rk[:], r(Wk_sb[:, k, :]), r(hT_sb[:, k, :]),
                start=(k == 0), stop=(k == KT - 1),
            )
        rkT_sb = const.tile([LOC, BC], f32)
        nc.scalar.activation(out=rkT_sb[:], in_=ps_rk[:], func=AF.Tanh,
                             bias=bk_sb[:], scale=1.0)

        # ------------- kn = rk / ||rk||  (per-sample norm) ------------------
        rksq_sb = const.tile([LOC, BC], f32)
        nc.vector.tensor_mul(rksq_sb[:], rkT_sb[:], rkT_sb[:])
        ps_ss = ps_misc.tile([1, BC], f32, tag="m")  # sum over 64 feats via ones-matmul
        nc.tensor.matmul(ps_ss[:], r(ones_sb[0:LOC, 0:1]), r(rksq_sb[:]),
                         start=True, stop=True)
        lnss_sb = const.tile([1, BC], f32)
        nc.scalar.activation(out=lnss_sb[:], in_=ps_ss[:], func=AF.Ln)
        invn_sb = const.tile([1, BC], f32)  # 1/||rk|| = exp(-0.5 ln(ss))
        nc.scalar.activation(out=invn_sb[:], in_=lnss_sb[:], func=AF.Exp,
                             scale=-0.5)
        # broadcast 1/||rk|| across the 64 feature partitions via K=1 matmul
        ps_bc = ps_misc.tile([LOC, BC], f32, tag="m")
        nc.tensor.matmul(ps_bc[:], r(ones_sb[0:1, 0:LOC]), r(invn_sb[:]),
                         start=True, stop=True)
        knT_sb = const.tile([LOC, BC], bf16)
        nc.vector.tensor_mul(knT_sb[:], rkT_sb[:], ps_bc[:])

        # ------------- streaming pass over the 65536 memory locations -------
        def tap(name_d, ap, shape):
            t = const.tile(shape, f32, tag="tap" + name_d.name)
            nc.vector.tensor_copy(out=t[:], in_=ap)
            nc.sync.dma_start(out=name_d[:], in_=t[:])

        if DEBUG_TAPS:
            tap(dbg_kn, knT_sb[:], [LOC, BC])
        wlast_sb = const.tile([128, NCHUNK], f32)   # unnormalized w of sample BC-1
        ps_rT = ps_r.tile([LOC + 1, BC], f32)       # rows 0..63: r^T; row 64: Z
        for sb in range(NSB):
            c0 = sb * SUPER
            mn_t = mnp.tile([LOC, SUPER, 128], bf16)
            nc.sync.dma_start(
                out=mn_t[:],
                in_=MnT[:, c0 * 128:(c0 + SUPER) * 128].rearrange(
                    "f (s c) -> f s c", s=SUPER),
            )
            mo_t = mop.tile([128, SUPER, LOC + 1], bf16)
            nc.sync.dma_start(out=mo_t[:], in_=Mo[:, c0:c0 + SUPER, :])
            w2_t = w2p.tile([128, SUPER, BC], bf16)
            if DEBUG_TAPS and sb == 0:
                tap(dbg_mn, mn_t[:, 0, :], [LOC, 128])
                tap(dbg_mo, mo_t[:, 0, :], [128, LOC + 1])
            for g in range(SUPER // GROUP):
                ps_s = ps_sim.tile([128, GROUP, BC], f32)
                for j in range(GROUP):
                    nc.tensor.matmul(
                        ps_s[:, j, :],
                        r(mn_t[:, g * GROUP + j, :]),
                        r(knT_sb[:]),
                        start=True, stop=True,
                    )
                if DEBUG_TAPS and sb == 0 and g == 0:
                    tap(dbg_sim, ps_s[:, 0, :], [128, BC])
                nc.scalar.activation(
                    out=w2_t[:, g * GROUP:(g + 1) * GROUP, :],
                    in_=ps_s[:], func=AF.Exp,
                )
                if DEBUG_TAPS and sb == 0 and g == 0:
                    tap(dbg_w2, w2_t[:, 0, :], [128, BC])
            for j in range(SUPER):
                c = c0 + j
                nc.tensor.matmul(
                    ps_rT[:], r(mo_t[:, j, :]), r(w2_t[:, j, :]),
                    start=(c == 0), stop=(c == NCHUNK - 1),
                )
            nc.vector.tensor_copy(out=wlast_sb[:, c0:c0 + SUPER],
                                  in_=w2_t[:, :, BC - 1])

        # ------------- epilogue ---------------------------------------------
        if DEBUG_TAPS:
            tap(dbg_racc, ps_rT[:], [LOC + 1, BC])
        # zw[64, b] = 1/Z_b  (stays on partition 64, where Z landed)
        zw_sb = const.tile([128, BC], f32)
        nc.vector.reciprocal(out=zw_sb[64:65, :], in_=ps_rT[LOC:LOC + 1, :])

        # out = hT^T WoH + (rT^T WoR) / Z + bo, all as [1, 256] rows
        racc_sb = const.tile([LOC, BC], f32)
        nc.vector.tensor_copy(out=racc_sb[:], in_=ps_rT[0:LOC, :])
        ps_oh = ps_misc.tile([1, BC], f32, tag="m")
        for k in range(CT):
            nc.tensor.matmul(ps_oh[:], r(WoH_sb[:, k:k + 1]), r(hT_sb[:, k, :]),
                             start=(k == 0), stop=(k == CT - 1))
        ps_or = ps_misc.tile([1, BC], f32, tag="m")
        nc.tensor.matmul(ps_or[:], r(WoR_sb[:]), r(racc_sb[:]),
                         start=True, stop=True)
        # broadcast 1/Z from partition 64 to partition 0 (row 0 of [64, BC])
        ps_zb = ps_misc.tile([LOC, BC], f32, tag="m")
        nc.tensor.matmul(ps_zb[:], r(ones_sb[64:65, 0:LOC]), r(zw_sb[64:65, :]),
                         start=True, stop=True)
        zb_sb = const.tile([LOC, BC], f32)
        nc.vector.tensor_copy(out=zb_sb[:], in_=ps_zb[:])
        t1_sb = const.tile([1, BC], f32)
        nc.vector.tensor_mul(t1_sb[:], zb_sb[0:1, :], ps_or[:])
        t2_sb = const.tile([1, BC], f32)
        nc.vector.tensor_add(t2_sb[:], t1_sb[:], ps_oh[:])
        outv_sb = const.tile([1, BC], f32)
        nc.vector.tensor_scalar_add(outv_sb[:], t2_sb[:], bo_sb[0:1, 0:1])
        nc.sync.dma_start(out=out_d[:], in_=outv_sb[:])

        # h[-1]: column BC-1 of hT
        hl_sb = const.tile([128, CT], f32)
        nc.vector.tensor_copy(out=hl_sb[:], in_=hT_sb[:, :, BC - 1])
        nc.sync.dma_start(out=hl_d[:], in_=hl_sb[:])

        # w_read[-1]: wlast * (1/Z[BC-1]) broadcast to all 128 partitions
        ps_zl = ps_misc.tile([128, 1], f32, tag="m")
        nc.tensor.matmul(ps_zl[:], r(ones_sb[64:65, :]),
                         r(zw_sb[64:65, BC - 1:BC]), start=True, stop=True)
        zl_sb = const.tile([128, 1], f32)
        nc.vector.tensor_copy(out=zl_sb[:], in_=ps_zl[:])
        wlf_sb = const.tile([128, NCHUNK], f32)
        nc.vector.tensor_scalar_mul(wlf_sb[:], wlast_sb[:], zl_sb[:, 0:1])
        nc.sync.dma_start(out=wl_d[:], in_=wlf_sb[:])

    nc.compile()
    return nc


def _get_program():
    if "nc" not in _CACHE:
        _CACHE["nc"] = _build_program()
    return _CACHE["nc"]


def _prep_in_maps(inputs):
    return _prep(**{k: np.asarray(v) for k, v in inputs.items()})


def _prep(x, W_h, b_h, W_g, b_g, W_k, b_k, M, W_out, b_out):
    x = np.ascontiguousarray(np.asarray(x, dtype=np.float32))
    W_h = np.asarray(W_h, dtype=np.float32)
    b_h = np.asarray(b_h, dtype=np.float32)
    W_g = np.asarray(W_g, dtype=np.float32)
    b_g = np.asarray(b_g, dtype=np.float32)
    W_k = np.asarray(W_k, dtype=np.float32)
    b_k = np.asarray(b_k, dtype=np.float32)
    M = np.ascontiguousarray(np.asarray(M, dtype=np.float32))
    W_out = np.asarray(W_out, dtype=np.float32)
    b_out = np.asarray(b_out, dtype=np.float32)

    # ---- host-side layout prep (weight swizzles for SBUF-friendly DMA) ----
    import ml_dtypes
    bf = ml_dtypes.bfloat16
    norms = np.linalg.norm(M, axis=1, keepdims=True)
    MnT = np.ascontiguousarray((M / (norms + EPS)).T).astype(bf)  # [64, 65536]
    Mo = np.concatenate([M, np.ones((N_LOC, 1), np.float32)], axis=1)
    Mo = np.ascontiguousarray(
        Mo.reshape(NCHUNK, 128, LOC + 1).transpose(1, 0, 2)).astype(bf)

    Wh_p = np.ascontiguousarray(W_h.reshape(KT, 128, CTRL).transpose(1, 0, 2))
    bh_p = np.ascontiguousarray(b_h.reshape(CT, 128).T)
    Wg_p = np.ascontiguousarray(W_g[:, 0].reshape(KT, 128).T)
    bg_p = b_g.reshape(1, 1)
    Wk_p = np.ascontiguousarray(W_k.reshape(KT, 128, LOC).transpose(1, 0, 2))
    bk_p = b_k.reshape(LOC, 1)
    WoH_p = np.ascontiguousarray(W_out[:CTRL, 0].reshape(CT, 128).T)
    WoR_p = np.ascontiguousarray(W_out[CTRL:, 0:1])
    bo_p = b_out.reshape(1, 1)

    shared = dict(Wh=Wh_p, bh=bh_p, Wg=Wg_p, bg=bg_p, Wk=Wk_p, bk=bk_p,
                  MnT=MnT, Mo=Mo, WoH=WoH_p, WoR=WoR_p, bo=bo_p)
    in_maps = []
    for c in range(NCORES):
        xc = x[c * BC:(c + 1) * BC]                              # [256, 512]
        xT_p = np.ascontiguousarray(
            xc.T.reshape(KT, 128, BC).transpose(1, 0, 2))        # [128, 4, 256]
        in_maps.append(dict(shared, xT=xT_p))
    return in_maps


def kernel(x, W_h, b_h, W_g, b_g, W_k, b_k, M, W_out, b_out):
    from concourse.bass_utils import run_bass_kernel_spmd

    in_maps = _prep(x, W_h, b_h, W_g, b_g, W_k, b_k, M, W_out, b_out)
    nc = _get_program()
    res = run_bass_kernel_spmd(nc, in_maps, core_ids=list(range(NCORES))).results

    output = np.concatenate([res[c]["out"][0] for c in range(NCORES)])
    h_last = np.ascontiguousarray(res[NCORES - 1]["hl"].T).reshape(CTRL)
    gate_last = res[NCORES - 1]["gt"][0, BC - 1:BC].copy()
    w_read_last = np.ascontiguousarray(res[NCORES - 1]["wl"].T).reshape(N_LOC)
    return (output.astype(np.float32), h_last.astype(np.float32),
            gate_last.astype(np.float32), w_read_last.astype(np.float32))
